# revision 49
# baseline (speedup 1.0000x reference)
"""Trainium2 Bass kernel for the BiaffineLayer problem (v3).

Math (per batch b):
  out[l, m, c] = x1[l] @ W1[c] + x2[m] @ W2[c]
              + sum_h x1[l,h] * x2[m,h] * W3[c,h]
              + sum_h |x1[l,h] - x2[m,h]| * W4[c,h] + bias[c]
  shapes: x1, x2 [2, 512, 128]; W [25, 512]; bias [25]; out [2, 512, 512, 25]

Sharding: 8 cores = 2 batches x 4 m-blocks of 128 columns. Each core gets the
full x1[b] (transposed to [h, l]) and its x2[b, m0:m0+128] block (transposed),
W/bias replicated, and produces out[b, :, m0:m0+128, :] = [512, 128, 25],
emitted as bf16 and upconverted to f32 on the host.

Changes vs the 66.9us baseline (measured on hw at each step):
  - all inputs arrive pre-cast to bf16 (no on-device casts; half the in-DMA).
  - bf16 output; psum->sbuf copies emit bf16 and one batched 3D DMA per
    ms-block writes 4 l-chunks at once (8 big out-DMAs + 4 tail ones).
  - D-tile generation is software-pipelined one ms-block ahead of the
    matmuls/copies so the in-order DVE/ACT queues never stall PE's next
    tiles behind a psum-dependent copy; split 15 DVE / 1 ACT: lightening
    ACT relieves SBUF port contention on x1, speeding DVE's tensor_scalar
    well below its contended ~360ns/tile rate.
  - bias matmul uses an all-ones [128,128] stationary (K=128 -> fast FWL
    LDWEIGHTS) against t2wide (row0 = t2row, rest zero) instead of a K=1
    ones row whose LDWEIGHTS can't use FWL.
  - bias+copy of each chunk lag two chunks behind its accumulation so the
    psum writeback drain overlaps useful PE work.
  - 12 tiny warmup matmuls ramp the PE p-state during the DMA prologue.
  - abs via |d| = 2*relu(d) - d with the -d part folded into V3'/T2B
    (unchanged from baseline).
"""

import sys

sys.path.insert(0, "/opt/trn_rl_repo")

from contextlib import ExitStack

import numpy as np

import concourse.bass as bass
import concourse.tile as tile
from concourse import bacc, bass_utils, mybir

F32 = mybir.dt.float32
BF16 = mybir.dt.bfloat16

B, L, H, C = 2, 512, 128, 25
MB = 128          # m-block per core
N_CORES = 8
MSUB = 16         # m's per psum chunk
N_MS = MB // MSUB  # 8 chunks over the m-block
LCHUNK = 128
N_LC = L // LCHUNK  # 4 l-chunks
CHUNK_F = MSUB * C  # 400 psum free columns per chunk
N_DVE_D = 13      # of each 16 D-tiles: this many on DVE, rest on ACT
DVE_COPY_LC = ()    # lc's whose psum->sbuf copy runs on DVE (rest on ACT)


def build_kernel(nc: bass.Bass):
    x1t = nc.dram_tensor("x1t", (H, L), BF16, kind="ExternalInput").ap()
    x2t = nc.dram_tensor("x2t", (H, MB), BF16, kind="ExternalInput").ap()
    wmov = nc.dram_tensor("wmov", (H, 4 * C), BF16, kind="ExternalInput").ap()
    brow = nc.dram_tensor("brow", (1, C), BF16, kind="ExternalInput").ap()
    x2tf = nc.dram_tensor("x2tf", (H, MB), F32, kind="ExternalInput").ap()
    id128 = nc.dram_tensor("id128", (H, H), BF16, kind="ExternalInput").ap()
    out = nc.dram_tensor("out", (L, MB * C), BF16, kind="ExternalOutput").ap()

    with tile.TileContext(nc) as tc, ExitStack() as ctx:
      const = ctx.enter_context(tc.tile_pool(name="const", bufs=1))
      dpool = ctx.enter_context(tc.tile_pool(name="dpool", bufs=40))
      opool = ctx.enter_context(tc.tile_pool(name="opool", bufs=4))
      psum = ctx.enter_context(tc.tile_pool(name="psum", bufs=7, space="PSUM"))
      psum_t2 = ctx.enter_context(tc.tile_pool(name="psum_t2", bufs=1, space="PSUM"))
      dram = ctx.enter_context(tc.tile_pool(name="dram", bufs=1, space="DRAM"))

      # ---- constant loads: all inputs arrive pre-cast to bf16 ----
      ones_bf = const.tile([1, MB], BF16)
      nc.vector.memset(ones_bf[:], 1.0)
      wscr = const.tile([1, C], BF16)
      nc.vector.memset(wscr[:], 0.125)
      # PE p-state warmup: tiny matmuls (no DMA dependency) ramp the clock to
      # 2.4GHz while DMA brings inputs in and DVE/ACT run the prologue.
      warm = psum_t2.tile([MB, C], F32, tag="t2")
      for _w in range(12):
          nc.tensor.matmul(warm[:], ones_bf[:], wscr[:],
                           start=True, stop=True, skip_group_check=True)
      x1t_bf = const.tile([H, L], BF16)
      nc.sync.dma_start(x1t_bf[:], x1t[:])
      id_bf = const.tile([H, H], BF16)
      nc.gpsimd.dma_start(id_bf[:], id128[:])
      x2t_bf = const.tile([H, MB], BF16)
      nc.sync.dma_start(x2t_bf[:], x2t[:])
      wmov_bf = const.tile([H, 4 * C], BF16)
      nc.sync.dma_start(wmov_bf[:], wmov[:])
      w1m4_bf = wmov_bf[:, 0:C]            # (W1-W4)T
      w2p4_bf = wmov_bf[:, C : 2 * C]      # (W2+W4)T
      w3t_bf = wmov_bf[:, 2 * C : 3 * C]   # W3T
      w4t_bf = wmov_bf[:, 3 * C : 4 * C]   # 2*W4T
      x2t_f = const.tile([H, MB], F32)
      nc.sync.dma_start(x2t_f[:], x2tf[:])
      negx2_f = const.tile([H, MB], F32)
      nc.vector.tensor_scalar_mul(negx2_f[:], x2t_f[:], -1.0)
      brow_bf = const.tile([1, C], BF16)
      nc.gpsimd.dma_start(brow_bf[:], brow[:])

      # ---- T2B row: t2[m, c] + bias[c] -> [1, MB*C] bf16 ----
      ps_t2 = psum_t2.tile([MB, C], F32, tag="t2")
      nc.tensor.matmul(ps_t2[:], x2t_bf[:], w2p4_bf,
                       start=True, stop=False, skip_group_check=True)
      nc.tensor.matmul(ps_t2[:], ones_bf[:], brow_bf[:],
                       start=False, stop=True, skip_group_check=True)
      t2small = const.tile([MB, C], F32)
      nc.scalar.copy(t2small[:], ps_t2[:])
      t2_dram = dram.tile([1, MB * C], F32)
      nc.sync.dma_start(t2_dram[:].rearrange("o (m c) -> (o m) c", c=C), t2small[:])
      t2row_bf = const.tile([1, MB * C], BF16)
      nc.gpsimd.dma_start(t2row_bf[:], t2_dram[:])  # casting DMA f32->bf16
      # t2wide: row 0 = t2row, rows 1..127 = 0. The bias matmul then uses the
      # all-ones [128,128] stationary (K=128 -> FWL-fast LDWEIGHTS) instead of
      # the K=1 ones row whose 128-col LDWEIGHTS can't use FWL.
      t2wide = const.tile([LCHUNK, MB * C], BF16)
      nc.gpsimd.memset(t2wide[:], 0.0)
      nc.gpsimd.dma_start(t2wide[0:1, :], t2_dram[:])
      ones128_bf = const.tile([LCHUNK, LCHUNK], BF16)
      nc.vector.memset(ones128_bf[:], 1.0)
      # stage x1 in a psum bank: ACT's D-gen then reads PSUM (faster access,
      # zero SBUF read contention with DVE's tensor_scalar stream)
      psum_x1 = psum_t2.tile([H, L], F32, tag="t2")
      nc.tensor.matmul(psum_x1[:, 0:256], id_bf[:], x1t_bf[:, 0:256],
                       start=True, stop=True, skip_group_check=True)
      nc.tensor.matmul(psum_x1[:, 256:512], id_bf[:], x1t_bf[:, 256:512],
                       start=True, stop=True, skip_group_check=True)


      # ---- V3[h, (m, c)] = x2t[h,m] * W3T[h,c] + (W1-W4)T[h,c]  (bf16) ----
      v3 = const.tile([H, MB * C], BF16)
      v3a = const.tile([H, MB * C], BF16)
      VS = 2 * MSUB  # V3 slice width in m's
      w3_bc = w3t_bf.unsqueeze(1).broadcast_to([H, VS, C])
      w1_bc = w1m4_bf.unsqueeze(1).broadcast_to([H, VS, C])

      def v3_prep(vh):
          sl = slice(vh * VS * C, (vh + 1) * VS * C)
          x2_bc = (x2t_bf[:, vh * VS : (vh + 1) * VS]
                   .unsqueeze(2).broadcast_to([H, VS, C]))
          v3a_3d = v3a[:, sl].rearrange("h (m c) -> h m c", c=C)
          nc.vector.tensor_tensor(v3a_3d, x2_bc, w3_bc, op=mybir.AluOpType.mult)
          nc.vector.tensor_tensor(v3[:, sl].rearrange("h (m c) -> h m c", c=C),
                                  v3a_3d, w1_bc, op=mybir.AluOpType.add)

      v3_prep(0)


      # ---- main loop: D-gen runs one ms ahead of the matmuls+copies so the
      # in-order DVE/ACT queues never block PE's next tiles behind a
      # psum-dependent copy. ----
      def gen_tiles(ms2):
          dts2 = []
          n_dve = 9 if ms2 == 0 else N_DVE_D
          for j2 in range(MSUB):
              m = ms2 * MSUB + j2
              dt_ = dpool.tile([H, L], BF16, tag="d")
              if j2 < n_dve:
                  nc.vector.tensor_scalar(
                      dt_[:], x1t_bf[:], x2t_f[:, m : m + 1], 0.0,
                      op0=mybir.AluOpType.subtract, op1=mybir.AluOpType.max)
              else:
                  nc.scalar.activation(
                      dt_[:], psum_x1[:], mybir.ActivationFunctionType.Relu,
                      bias=negx2_f[:, m : m + 1], scale=1.0)
              dts2.append(dt_)
          return dts2

      pending = []  # [(ps, ms, lc, o_sb), ...] chunks awaiting bias+copy
      LAG = 2

      def finish_chunk(ps, pms, plc, po_sb):
          nc.tensor.matmul(
              ps[:], ones128_bf[:],
              t2wide[:, pms * CHUNK_F : (pms + 1) * CHUNK_F],
              start=False, stop=True, skip_group_check=True)
          osl = po_sb[:, plc * CHUNK_F : (plc + 1) * CHUNK_F]
          on_dve = (plc in DVE_COPY_LC) or (pms == N_MS - 1 and plc % 2 == 0)
          if on_dve:
              nc.vector.tensor_copy(osl, ps[:])
          else:
              nc.scalar.copy(osl, ps[:])
          if pms == N_MS - 1:
              # drain the tail: per-chunk DMA instead of per-ms
              dst = (out[plc * LCHUNK : (plc + 1) * LCHUNK,
                         pms * CHUNK_F : (pms + 1) * CHUNK_F])
              nc.sync.dma_start(dst, osl)

      next_dts = gen_tiles(0)
      o_sbs2 = {}
      for ms in range(N_MS):
          dts = next_dts
          if ms % 2 == 0 and ms + 2 < N_MS:
              v3_prep(ms // 2 + 1)
          if ms + 1 < N_MS:
              next_dts = gen_tiles(ms + 1)
          o_sb = opool.tile([LCHUNK, N_LC * CHUNK_F], BF16, tag="o")
          o_sbs2[ms] = o_sb
          for lc in range(N_LC):
              ps = psum.tile([LCHUNK, CHUNK_F], F32, tag="ps")
              nc.tensor.matmul(
                  ps[:],
                  x1t_bf[:, lc * LCHUNK : (lc + 1) * LCHUNK],
                  v3[:, ms * CHUNK_F : (ms + 1) * CHUNK_F],
                  start=True, stop=False, skip_group_check=True)
              for j in range(MSUB):
                  nc.tensor.matmul(
                      ps[:, j * C : (j + 1) * C],
                      dts[j][:, lc * LCHUNK : (lc + 1) * LCHUNK],
                      w4t_bf,
                      start=False, stop=False,
                      skip_group_check=True)
              pending.append((ps, ms, lc, o_sb))
              if len(pending) > LAG:
                  finish_chunk(*pending.pop(0))
          if ms > 1:
              dst = (out[:, (ms - 2) * CHUNK_F : (ms - 1) * CHUNK_F]
                     .rearrange("(l p) f -> p l f", l=N_LC))
              nc.sync.dma_start(dst, o_sbs2[ms - 2][:]
                                .rearrange("p (l f) -> p l f", l=N_LC))
      while pending:
          finish_chunk(*pending.pop(0))
      dst = (out[:, (N_MS - 2) * CHUNK_F : (N_MS - 1) * CHUNK_F]
             .rearrange("(l p) f -> p l f", l=N_LC))
      nc.sync.dma_start(dst, o_sbs2[N_MS - 2][:]
                        .rearrange("p (l f) -> p l f", l=N_LC))
    return nc


_COMPILED = {}


def _get_compiled():
    if "nc" not in _COMPILED:
        nc = bacc.Bacc("TRN2", target_bir_lowering=False, debug=False,
                       num_devices=N_CORES)
        build_kernel(nc)
        nc.compile()
        _COMPILED["nc"] = nc
    return _COMPILED["nc"]


def make_in_maps(x1, x2, W, b):
    import ml_dtypes
    bf = ml_dtypes.bfloat16
    W1, W2, W3, W4 = (W[:, 0:H], W[:, H : 2 * H], W[:, 2 * H : 3 * H],
                      W[:, 3 * H : 4 * H])
    wmov = np.ascontiguousarray(
        np.concatenate([(W1 - W4).T, (W2 + W4).T, W3.T, (2.0 * W4).T],
                       axis=1)).astype(bf)
    brow = np.ascontiguousarray(b.reshape(1, C)).astype(bf)
    in_maps = []
    for cid in range(N_CORES):
        bb, mblk = cid // 4, cid % 4
        m0 = mblk * MB
        ident = np.eye(H).astype(bf)
        in_maps.append({
            "id128": ident,
            "x1t": np.ascontiguousarray(x1[bb].T).astype(bf),
            "x2t": np.ascontiguousarray(x2[bb, m0 : m0 + MB].T).astype(bf),
            "x2tf": np.ascontiguousarray(x2[bb, m0 : m0 + MB].T,
                                         dtype=np.float32),
            "wmov": wmov,
            "brow": brow,
        })
    return in_maps


def run_on_device(x1, x2, W, b, trace=False, trace_kwargs=None):
    nc = _get_compiled()
    in_maps = make_in_maps(x1, x2, W, b)
    res = bass_utils.run_bass_kernel_spmd(
        nc, in_maps, core_ids=list(range(N_CORES)), trace=trace,
        **(trace_kwargs or {}))
    full = np.empty((B, L, L, C), dtype=np.float32)
    for cid in range(N_CORES):
        bb, mblk = cid // 4, cid % 4
        m0 = mblk * MB
        full[bb, :, m0 : m0 + MB, :] = np.asarray(
            res.results[cid]["out"], dtype=np.float32).reshape(L, MB, C)
    return full, res


def kernel(x1, x2, W, b):
    x1 = np.asarray(x1, dtype=np.float32)
    x2 = np.asarray(x2, dtype=np.float32)
    W = np.asarray(W, dtype=np.float32)
    b = np.asarray(b, dtype=np.float32)
    full, _ = run_on_device(x1, x2, W, b, trace=False)
    return full


# revision 50
# speedup vs baseline: 1.2705x; 1.2705x over previous
"""Trainium2 Bass kernel for the BiaffineLayer problem (v3).

Math (per batch b):
  out[l, m, c] = x1[l] @ W1[c] + x2[m] @ W2[c]
              + sum_h x1[l,h] * x2[m,h] * W3[c,h]
              + sum_h |x1[l,h] - x2[m,h]| * W4[c,h] + bias[c]
  shapes: x1, x2 [2, 512, 128]; W [25, 512]; bias [25]; out [2, 512, 512, 25]

Sharding: 8 cores = 2 batches x 4 m-blocks of 128 columns. Each core gets the
full x1[b] (transposed to [h, l]) and its x2[b, m0:m0+128] block (transposed),
W/bias replicated, and produces out[b, :, m0:m0+128, :] = [512, 128, 25],
emitted as bf16 and upconverted to f32 on the host.

Changes vs the 66.9us baseline (measured on hw at each step):
  - all inputs arrive pre-cast to bf16 (no on-device casts; half the in-DMA).
  - bf16 output; psum->sbuf copies emit bf16 and one batched 3D DMA per
    ms-block writes 4 l-chunks at once (8 big out-DMAs + 4 tail ones).
  - D-tile generation is software-pipelined one ms-block ahead of the
    matmuls/copies so the in-order DVE/ACT queues never stall PE's next
    tiles behind a psum-dependent copy; split 15 DVE / 1 ACT: lightening
    ACT relieves SBUF port contention on x1, speeding DVE's tensor_scalar
    well below its contended ~360ns/tile rate.
  - bias matmul uses an all-ones [128,128] stationary (K=128 -> fast FWL
    LDWEIGHTS) against t2wide (row0 = t2row, rest zero) instead of a K=1
    ones row whose LDWEIGHTS can't use FWL.
  - bias+copy of each chunk lag two chunks behind its accumulation so the
    psum writeback drain overlaps useful PE work.
  - 12 tiny warmup matmuls ramp the PE p-state during the DMA prologue.
  - abs via |d| = 2*relu(d) - d with the -d part folded into V3'/T2B
    (unchanged from baseline).
"""

import sys

sys.path.insert(0, "/opt/trn_rl_repo")

from contextlib import ExitStack

import numpy as np

import concourse.bass as bass
import concourse.tile as tile
from concourse import bacc, bass_utils, mybir

F32 = mybir.dt.float32
BF16 = mybir.dt.bfloat16

B, L, H, C = 2, 512, 128, 25
MB = 128          # m-block per core
N_CORES = 8
MSUB = 16         # m's per psum chunk
N_MS = MB // MSUB  # 8 chunks over the m-block
LCHUNK = 128
N_LC = L // LCHUNK  # 4 l-chunks
CHUNK_F = MSUB * C  # 400 psum free columns per chunk
N_DVE_D = 15      # of each 16 D-tiles: this many on DVE, rest on ACT
DVE_COPY_LC = ()    # lc's whose psum->sbuf copy runs on DVE (rest on ACT)


def build_kernel(nc: bass.Bass):
    x1t = nc.dram_tensor("x1t", (H, L), BF16, kind="ExternalInput").ap()
    x2t = nc.dram_tensor("x2t", (H, MB), BF16, kind="ExternalInput").ap()
    wmov = nc.dram_tensor("wmov", (H, 4 * C), BF16, kind="ExternalInput").ap()
    brow = nc.dram_tensor("brow", (1, C), BF16, kind="ExternalInput").ap()
    x2tf = nc.dram_tensor("x2tf", (H, MB), F32, kind="ExternalInput").ap()
    out = nc.dram_tensor("out", (L, MB * C), BF16, kind="ExternalOutput").ap()

    with tile.TileContext(nc) as tc, ExitStack() as ctx:
      const = ctx.enter_context(tc.tile_pool(name="const", bufs=1))
      dpool = ctx.enter_context(tc.tile_pool(name="dpool", bufs=40))
      opool = ctx.enter_context(tc.tile_pool(name="opool", bufs=4))
      psum = ctx.enter_context(tc.tile_pool(name="psum", bufs=7, space="PSUM"))
      psum_t2 = ctx.enter_context(tc.tile_pool(name="psum_t2", bufs=1, space="PSUM"))
      dram = ctx.enter_context(tc.tile_pool(name="dram", bufs=1, space="DRAM"))

      # ---- constant loads: all inputs arrive pre-cast to bf16 ----
      ones_bf = const.tile([1, MB], BF16)
      nc.vector.memset(ones_bf[:], 1.0)
      wscr = const.tile([1, C], BF16)
      nc.vector.memset(wscr[:], 0.125)
      # PE p-state warmup: tiny matmuls (no DMA dependency) ramp the clock to
      # 2.4GHz while DMA brings inputs in and DVE/ACT run the prologue.
      warm = psum_t2.tile([MB, C], F32, tag="t2")
      for _w in range(12):
          nc.tensor.matmul(warm[:], ones_bf[:], wscr[:],
                           start=True, stop=True, skip_group_check=True)
      x1t_bf = const.tile([H, L], BF16)
      nc.sync.dma_start(x1t_bf[:], x1t[:])
      x1t_bf2 = const.tile([H, L], BF16)   # ACT's private copy (SBUF port
      nc.gpsimd.dma_start(x1t_bf2[:], x1t[:])  # contention relief)
      x2t_bf = const.tile([H, MB], BF16)
      nc.sync.dma_start(x2t_bf[:], x2t[:])
      wmov_bf = const.tile([H, 4 * C], BF16)
      nc.sync.dma_start(wmov_bf[:], wmov[:])
      w1m4_bf = wmov_bf[:, 0:C]            # (W1-W4)T
      w2p4_bf = wmov_bf[:, C : 2 * C]      # (W2+W4)T
      w3t_bf = wmov_bf[:, 2 * C : 3 * C]   # W3T
      w4t_bf = wmov_bf[:, 3 * C : 4 * C]   # 2*W4T
      x2t_f = const.tile([H, MB], F32)
      nc.sync.dma_start(x2t_f[:], x2tf[:])
      negx2_f = const.tile([H, MB], F32)
      nc.vector.tensor_scalar_mul(negx2_f[:], x2t_f[:], -1.0)
      brow_bf = const.tile([1, C], BF16)
      nc.gpsimd.dma_start(brow_bf[:], brow[:])

      # ---- T2B row: t2[m, c] + bias[c] -> [1, MB*C] bf16 ----
      ps_t2 = psum_t2.tile([MB, C], F32, tag="t2")
      nc.tensor.matmul(ps_t2[:], x2t_bf[:], w2p4_bf,
                       start=True, stop=False, skip_group_check=True)
      nc.tensor.matmul(ps_t2[:], ones_bf[:], brow_bf[:],
                       start=False, stop=True, skip_group_check=True)
      t2small = const.tile([MB, C], F32)
      nc.scalar.copy(t2small[:], ps_t2[:])
      t2_dram = dram.tile([1, MB * C], F32)
      nc.sync.dma_start(t2_dram[:].rearrange("o (m c) -> (o m) c", c=C), t2small[:])
      t2row_bf = const.tile([1, MB * C], BF16)
      nc.gpsimd.dma_start(t2row_bf[:], t2_dram[:])  # casting DMA f32->bf16
      # t2wide: row 0 = t2row, rows 1..127 = 0. The bias matmul then uses the
      # all-ones [128,128] stationary (K=128 -> FWL-fast LDWEIGHTS) instead of
      # the K=1 ones row whose 128-col LDWEIGHTS can't use FWL.
      t2wide = const.tile([LCHUNK, MB * C], BF16)
      nc.gpsimd.memset(t2wide[:], 0.0)
      nc.gpsimd.dma_start(t2wide[0:1, :], t2_dram[:])
      ones128_bf = const.tile([LCHUNK, LCHUNK], BF16)
      nc.vector.memset(ones128_bf[:], 1.0)


      # ---- V3[h, (m, c)] = x2t[h,m] * W3T[h,c] + (W1-W4)T[h,c]  (bf16) ----
      v3 = const.tile([H, MB * C], BF16)
      v3a = const.tile([H, MB * C], BF16)
      VS = 2 * MSUB  # V3 slice width in m's
      w3_bc = w3t_bf.unsqueeze(1).broadcast_to([H, VS, C])
      w1_bc = w1m4_bf.unsqueeze(1).broadcast_to([H, VS, C])

      def v3_prep(vh):
          sl = slice(vh * VS * C, (vh + 1) * VS * C)
          x2_bc = (x2t_bf[:, vh * VS : (vh + 1) * VS]
                   .unsqueeze(2).broadcast_to([H, VS, C]))
          v3a_3d = v3a[:, sl].rearrange("h (m c) -> h m c", c=C)
          nc.vector.tensor_tensor(v3a_3d, x2_bc, w3_bc, op=mybir.AluOpType.mult)
          nc.vector.tensor_tensor(v3[:, sl].rearrange("h (m c) -> h m c", c=C),
                                  v3a_3d, w1_bc, op=mybir.AluOpType.add)

      v3_prep(0)


      # ---- main loop: D-gen runs one ms ahead of the matmuls+copies so the
      # in-order DVE/ACT queues never block PE's next tiles behind a
      # psum-dependent copy. ----
      def gen_tiles(ms2):
          dts2 = []
          n_dve = 9 if ms2 == 0 else N_DVE_D
          for j2 in range(MSUB):
              m = ms2 * MSUB + j2
              dt_ = dpool.tile([H, L], BF16, tag="d")
              if j2 < n_dve:
                  nc.vector.tensor_scalar(
                      dt_[:], x1t_bf[:], x2t_f[:, m : m + 1], 0.0,
                      op0=mybir.AluOpType.subtract, op1=mybir.AluOpType.max)
              else:
                  nc.scalar.activation(
                      dt_[:], x1t_bf2[:], mybir.ActivationFunctionType.Relu,
                      bias=negx2_f[:, m : m + 1], scale=1.0)
              dts2.append(dt_)
          return dts2

      pending = []  # [(ps, ms, lc, o_sb), ...] chunks awaiting bias+copy
      LAG = 2

      def finish_chunk(ps, pms, plc, po_sb):
          nc.tensor.matmul(
              ps[:], ones128_bf[:],
              t2wide[:, pms * CHUNK_F : (pms + 1) * CHUNK_F],
              start=False, stop=True, skip_group_check=True)
          osl = po_sb[:, plc * CHUNK_F : (plc + 1) * CHUNK_F]
          on_dve = (plc in DVE_COPY_LC) or (pms == N_MS - 1 and plc % 2 == 0)
          if on_dve:
              nc.vector.tensor_copy(osl, ps[:])
          else:
              nc.scalar.copy(osl, ps[:])
          if pms == N_MS - 1:
              # drain the tail: per-chunk DMA instead of per-ms
              dst = (out[plc * LCHUNK : (plc + 1) * LCHUNK,
                         pms * CHUNK_F : (pms + 1) * CHUNK_F])
              nc.sync.dma_start(dst, osl)

      next_dts = gen_tiles(0)
      o_sbs2 = {}
      for ms in range(N_MS):
          dts = next_dts
          if ms % 2 == 0 and ms + 2 < N_MS:
              v3_prep(ms // 2 + 1)
          if ms + 1 < N_MS:
              next_dts = gen_tiles(ms + 1)
          o_sb = opool.tile([LCHUNK, N_LC * CHUNK_F], BF16, tag="o")
          o_sbs2[ms] = o_sb
          for lc in range(N_LC):
              ps = psum.tile([LCHUNK, CHUNK_F], F32, tag="ps")
              nc.tensor.matmul(
                  ps[:],
                  x1t_bf[:, lc * LCHUNK : (lc + 1) * LCHUNK],
                  v3[:, ms * CHUNK_F : (ms + 1) * CHUNK_F],
                  start=True, stop=False, skip_group_check=True)
              for j in range(MSUB):
                  nc.tensor.matmul(
                      ps[:, j * C : (j + 1) * C],
                      dts[j][:, lc * LCHUNK : (lc + 1) * LCHUNK],
                      w4t_bf,
                      start=False, stop=False,
                      skip_group_check=True)
              pending.append((ps, ms, lc, o_sb))
              if len(pending) > LAG:
                  finish_chunk(*pending.pop(0))
          if ms > 1:
              dst = (out[:, (ms - 2) * CHUNK_F : (ms - 1) * CHUNK_F]
                     .rearrange("(l p) f -> p l f", l=N_LC))
              nc.sync.dma_start(dst, o_sbs2[ms - 2][:]
                                .rearrange("p (l f) -> p l f", l=N_LC))
      while pending:
          finish_chunk(*pending.pop(0))
      dst = (out[:, (N_MS - 2) * CHUNK_F : (N_MS - 1) * CHUNK_F]
             .rearrange("(l p) f -> p l f", l=N_LC))
      nc.sync.dma_start(dst, o_sbs2[N_MS - 2][:]
                        .rearrange("p (l f) -> p l f", l=N_LC))
    return nc


_COMPILED = {}


def _get_compiled():
    if "nc" not in _COMPILED:
        nc = bacc.Bacc("TRN2", target_bir_lowering=False, debug=False,
                       num_devices=N_CORES)
        build_kernel(nc)
        nc.compile()
        _COMPILED["nc"] = nc
    return _COMPILED["nc"]


def make_in_maps(x1, x2, W, b):
    import ml_dtypes
    bf = ml_dtypes.bfloat16
    W1, W2, W3, W4 = (W[:, 0:H], W[:, H : 2 * H], W[:, 2 * H : 3 * H],
                      W[:, 3 * H : 4 * H])
    wmov = np.ascontiguousarray(
        np.concatenate([(W1 - W4).T, (W2 + W4).T, W3.T, (2.0 * W4).T],
                       axis=1)).astype(bf)
    brow = np.ascontiguousarray(b.reshape(1, C)).astype(bf)
    in_maps = []
    for cid in range(N_CORES):
        bb, mblk = cid // 4, cid % 4
        m0 = mblk * MB
        in_maps.append({
            "x1t": np.ascontiguousarray(x1[bb].T).astype(bf),
            "x2t": np.ascontiguousarray(x2[bb, m0 : m0 + MB].T).astype(bf),
            "x2tf": np.ascontiguousarray(x2[bb, m0 : m0 + MB].T,
                                         dtype=np.float32),
            "wmov": wmov,
            "brow": brow,
        })
    return in_maps


def run_on_device(x1, x2, W, b, trace=False, trace_kwargs=None):
    nc = _get_compiled()
    in_maps = make_in_maps(x1, x2, W, b)
    res = bass_utils.run_bass_kernel_spmd(
        nc, in_maps, core_ids=list(range(N_CORES)), trace=trace,
        **(trace_kwargs or {}))
    full = np.empty((B, L, L, C), dtype=np.float32)
    for cid in range(N_CORES):
        bb, mblk = cid // 4, cid % 4
        m0 = mblk * MB
        full[bb, :, m0 : m0 + MB, :] = np.asarray(
            res.results[cid]["out"], dtype=np.float32).reshape(L, MB, C)
    return full, res


def kernel(x1, x2, W, b):
    x1 = np.asarray(x1, dtype=np.float32)
    x2 = np.asarray(x2, dtype=np.float32)
    W = np.asarray(W, dtype=np.float32)
    b = np.asarray(b, dtype=np.float32)
    full, _ = run_on_device(x1, x2, W, b, trace=False)
    return full
